# revision 11
# baseline (speedup 1.0000x reference)
"""CfC (closed-form continuous-time) cell kernel for Trainium2, 8 NeuronCores.

Reference computation (B=8192, IN=256, H=512, all fp32):
    g     = sigmoid(x @ W_gx.T + b_gx + h @ W_gh.T + gate_b)        [B, H]
    f     = tanh(cat([x, h]) @ W_backbone.T + b_backbone)           [B, H]
    tau   = softplus(log_tau) + |g|          (g in (0,1) so |g| == g)
    decay = exp(-delta_t[:, None] * tau)
    out   = decay * h + (1 - decay) * f

Strategy: data-parallel over B (1024 rows per core), weights replicated.
All device work happens in "feature-major" (transposed) layout: activations
are shipped as xh^T [768, B_shard] so the contraction dim lands on SBUF
partitions with no on-device transposes.  Gate and backbone share the same
moving operand (xh^T); their transposed weight matrices are stacked into one
[768, 1024] tensor.  Matmul inputs are cast to fp16 on the host (halves the
input stream and runs the PE at full rate with fast weight load; fp16's
10-bit mantissa keeps matmul error ~4x below bf16 and the operands are far
from fp16 range limits); h is also
shipped in fp32 for the elementwise decay mix, which runs in fp32.
Elementwise uses the transposed layout where per-feature vectors (biases,
softplus(log_tau)) are per-partition scalars that fuse into ACT bias/scale
slots, and delta_t is broadcast along partitions once by a step-0 DMA.
sigmoid(u) is computed as 0.5 + 0.5*tanh(u/2) because Sigmoid and Exp never
share an ACT table (the 0.5s fold into existing affine slots); softplus(x) is
ln(1+exp(x)) for the same reason.  The [512, B_shard] per-core results are
gathered and transposed on the host.
"""

from contextlib import ExitStack

import ml_dtypes
import numpy as np

import concourse.bass as bass
import concourse.mybir as mybir
import concourse.tile as tile
from concourse import bacc
from concourse.bass_utils import run_bass_kernel_spmd

B, IN, H = 8192, 256, 512
NCORES = 8
BS = B // NCORES          # 1024 batch rows per core
KIN = IN + H              # 768 contraction dim
KT = KIN // 128           # 6 k-tiles
NJ = H // 128             # 4 partition tiles per output matrix
NCHUNK = 512              # matmul moving free dim per PSUM bank
NCH = BS // NCHUNK        # 2 b-chunks per core

F32 = mybir.dt.float32
F32R = mybir.dt.float32r
FP16 = mybir.dt.float16
AF = mybir.ActivationFunctionType
OP = mybir.AluOpType

USE_FP16 = True           # fp16 matmul inputs (2x PE rate + FWL) vs fp32r
TRACE = False             # test.py flips this for profiled runs
LAST_RESULT = None        # BassKernelResults of the most recent run

_NC_CACHE = None


def _body(tc, xhT, WT, hT, consts, negdt, outT):
    nc = tc.nc
    mm_dt = FP16 if USE_FP16 else F32
    with ExitStack() as ctx:
        singles = ctx.enter_context(tc.tile_pool(name="singles", bufs=1))
        work = ctx.enter_context(tc.tile_pool(name="work", bufs=3))
        psg = ctx.enter_context(tc.tile_pool(name="psg", bufs=4, space="PSUM"))
        psf = ctx.enter_context(tc.tile_pool(name="psf", bufs=4, space="PSUM"))

        # Persistent SBUF tensors
        xh_sb = singles.tile([128, KT, BS], mm_dt, tag="xh")
        w_sb = singles.tile([128, KT, 2 * H], mm_dt, tag="w")
        h_sb = singles.tile([128, NJ, BS], F32, tag="h")
        cst = singles.tile([128, 3, NJ], F32, tag="cst")
        stau = singles.tile([128, NJ], F32, tag="stau")
        ndt = singles.tile([128, BS], F32, tag="ndt")

        xh_v = xhT.rearrange("(k p) b -> k p b", p=128)
        w_v = WT.rearrange("(k p) n -> k p n", p=128)
        h_v = hT.rearrange("(j p) b -> j p b", p=128)

        # consts is [bg/2 | bb | log_tau] each [H]; column j of the SBUF tile
        # is that vector's j-th 128-row slice (per-partition scalars).
        nc.sync.dma_start(
            out=cst, in_=consts.rearrange("(c j p) -> p c j", p=128, j=NJ)
        )
        # -delta_t broadcast to all 128 partitions (step-0 partition dim)
        nc.sync.dma_start(
            out=ndt,
            in_=bass.AP(
                tensor=negdt.tensor,
                offset=negdt.offset,
                ap=[[0, 128], negdt.ap[0]],
            ),
        )
        # softplus(log_tau) = ln(1 + exp(log_tau)); the ACT table with
        # Softplus itself never ships with Exp/Tanh, so build it from Ln/Exp.
        # stau holds softplus(log_tau) + 0.5 — the 0.5 is the constant term of
        # sigmoid(u) = 0.5 + 0.5*tanh(u/2), folded into tau below.
        e0 = singles.tile([128, NJ], F32, tag="e0")
        nc.scalar.activation(out=e0, in_=cst[:, 2, :], func=AF.Exp)
        nc.vector.tensor_scalar_add(e0, e0, 1.0)
        nc.scalar.activation(out=stau, in_=e0, func=AF.Ln)
        nc.vector.tensor_scalar_add(stau, stau, 0.5)

        # Stream inputs in matmul consumption order (k-major).
        def mm_cast(ap):
            return ap if USE_FP16 else ap.bitcast(F32R)

        for k in range(KT):
            nc.sync.dma_start(out=mm_cast(w_sb[:, k, :]), in_=mm_cast(w_v[k]))
            nc.sync.dma_start(out=mm_cast(xh_sb[:, k, :]), in_=mm_cast(xh_v[k]))
        for j in range(NJ):
            nc.sync.dma_start(out=h_sb[:, j, :], in_=h_v[j])

        for n in range(NCH):
            bsl = slice(n * NCHUNK, (n + 1) * NCHUNK)
            zgs = [psg.tile([128, NCHUNK], F32, tag="zg", name=f"zg_{n}_{j}") for j in range(NJ)]
            zfs = [psf.tile([128, NCHUNK], F32, tag="zf", name=f"zf_{n}_{j}") for j in range(NJ)]
            # k-outer so PE consumes each streamed k-tile for all 8 output
            # groups before needing the next one.
            for k in range(KT):
                rhs = mm_cast(xh_sb[:, k, bsl])
                for j in range(NJ):
                    nc.tensor.matmul(
                        zgs[j],
                        mm_cast(w_sb[:, k, j * 128:(j + 1) * 128]),
                        rhs,
                        start=(k == 0),
                        stop=(k == KT - 1),
                    )
                for j in range(NJ):
                    nc.tensor.matmul(
                        zfs[j],
                        mm_cast(w_sb[:, k, H + j * 128:H + (j + 1) * 128]),
                        rhs,
                        start=(k == 0),
                        stop=(k == KT - 1),
                    )
            for j in range(NJ):
                tg = work.tile([128, NCHUNK], F32, tag="tg", name=f"tg_{n}_{j}")
                f = work.tile([128, NCHUNK], F32, tag="f", name=f"f_{n}_{j}")
                tau = work.tile([128, NCHUNK], F32, tag="tau", name=f"tau_{n}_{j}")
                t = work.tile([128, NCHUNK], F32, tag="t", name=f"t_{n}_{j}")
                decay = work.tile([128, NCHUNK], F32, tag="decay", name=f"decay_{n}_{j}")
                hmf = work.tile([128, NCHUNK], F32, tag="hmf", name=f"hmf_{n}_{j}")
                p = work.tile([128, NCHUNK], F32, tag="p", name=f"p_{n}_{j}")
                o = work.tile([128, NCHUNK], F32, tag="o", name=f"o_{n}_{j}")

                # sigmoid(zg + bg) = 0.5 + 0.5*tanh((zg + bg)/2): Sigmoid never
                # shares an ACT table with Exp, but Tanh does.  cst slot 0
                # holds bg/2; the 0.5 offsets are folded into stau.
                nc.scalar.activation(
                    out=tg, in_=zgs[j], func=AF.Tanh, bias=cst[:, 0, j:j + 1],
                    scale=0.5,
                )
                nc.scalar.activation(
                    out=f, in_=zfs[j], func=AF.Tanh, bias=cst[:, 1, j:j + 1]
                )
                # tau = g + softplus(log_tau) = 0.5*tg + (softplus+0.5)
                nc.vector.tensor_scalar(
                    out=tau, in0=tg, scalar1=0.5, scalar2=stau[:, j:j + 1],
                    op0=OP.mult, op1=OP.add,
                )
                # t = -dt * tau
                nc.vector.tensor_mul(out=t, in0=tau, in1=ndt[:, bsl])
                nc.scalar.activation(out=decay, in_=t, func=AF.Exp)
                # out = f + decay * (h - f)
                nc.gpsimd.tensor_sub(out=hmf, in0=h_sb[:, j, bsl], in1=f)
                nc.vector.tensor_mul(out=p, in0=decay, in1=hmf)
                nc.vector.tensor_add(out=o, in0=p, in1=f)
                nc.sync.dma_start(
                    out=outT[j * 128:(j + 1) * 128, bsl], in_=o
                )


def build_nc():
    mm_np_dt = FP16 if USE_FP16 else F32
    nc = bacc.Bacc(
        "TRN2",
        target_bir_lowering=False,
        debug=False,
        enable_asserts=False,
        num_devices=NCORES,
    )
    xhT = nc.dram_tensor("xhT", [KIN, BS], mm_np_dt, kind="ExternalInput").ap()
    WT = nc.dram_tensor("WT", [KIN, 2 * H], mm_np_dt, kind="ExternalInput").ap()
    hT = nc.dram_tensor("hT", [H, BS], F32, kind="ExternalInput").ap()
    consts = nc.dram_tensor("consts", [3 * H], F32, kind="ExternalInput").ap()
    negdt = nc.dram_tensor("negdt", [BS], F32, kind="ExternalInput").ap()
    outT = nc.dram_tensor("outT", [H, BS], F32, kind="ExternalOutput").ap()
    with tile.TileContext(nc) as tc:
        _body(tc, xhT, WT, hT, consts, negdt, outT)
    nc.compile()
    return nc


def _get_nc():
    global _NC_CACHE
    if _NC_CACHE is None:
        _NC_CACHE = build_nc()
    return _NC_CACHE


def make_in_maps(x, h, delta_t, W_backbone, b_backbone, W_gx, b_gx, W_gh,
                 gate_b, log_tau):
    f32 = np.float32
    mm_dt = np.float16 if USE_FP16 else f32
    xh = np.concatenate(
        [np.asarray(x, f32), np.asarray(h, f32)], axis=1
    )                                                   # [B, 768]
    xhT = np.ascontiguousarray(xh.T)                    # [768, B]
    hT = xhT[IN:]                                       # [512, B] fp32 view
    WgT = np.concatenate(
        [np.asarray(W_gx, f32), np.asarray(W_gh, f32)], axis=1
    ).T                                                 # [768, H]
    WT = np.concatenate(
        [WgT, np.asarray(W_backbone, f32).T], axis=1
    ).astype(mm_dt)                                     # [768, 2H]
    WT = np.ascontiguousarray(WT)
    xhT_mm = xhT.astype(mm_dt)
    consts = np.concatenate(
        [
            (np.asarray(b_gx, f32) + np.asarray(gate_b, f32)) * 0.5,
            np.asarray(b_backbone, f32),
            np.asarray(log_tau, f32),
        ]
    ).astype(f32)                                       # [3H]
    negdt = (-np.asarray(delta_t, f32)).astype(f32)     # [B]

    in_maps = []
    for c in range(NCORES):
        sl = slice(c * BS, (c + 1) * BS)
        in_maps.append(
            {
                "xhT": np.ascontiguousarray(xhT_mm[:, sl]),
                "WT": WT,
                "hT": np.ascontiguousarray(hT[:, sl]),
                "consts": consts,
                "negdt": np.ascontiguousarray(negdt[sl]),
            }
        )
    return in_maps


def kernel(x, h, delta_t, W_backbone, b_backbone, W_gx, b_gx, W_gh, gate_b,
           log_tau):
    global LAST_RESULT
    in_maps = make_in_maps(x, h, delta_t, W_backbone, b_backbone, W_gx, b_gx,
                           W_gh, gate_b, log_tau)
    nc = _get_nc()
    res = run_bass_kernel_spmd(
        nc, in_maps, core_ids=list(range(NCORES)), trace=TRACE
    )
    LAST_RESULT = res
    out = np.concatenate([r["outT"] for r in res.results], axis=1).T
    return np.ascontiguousarray(out).astype(np.float32)


# revision 14
# speedup vs baseline: 1.0069x; 1.0069x over previous
"""CfC (closed-form continuous-time) cell kernel for Trainium2, 8 NeuronCores.

Reference computation (B=8192, IN=256, H=512, all fp32):
    g     = sigmoid(x @ W_gx.T + b_gx + h @ W_gh.T + gate_b)        [B, H]
    f     = tanh(cat([x, h]) @ W_backbone.T + b_backbone)           [B, H]
    tau   = softplus(log_tau) + |g|          (g in (0,1) so |g| == g)
    decay = exp(-delta_t[:, None] * tau)
    out   = decay * h + (1 - decay) * f

Strategy: data-parallel over B (1024 rows per core), weights replicated.
All device work happens in "feature-major" (transposed) layout: activations
are shipped as xh^T [768, B_shard] so the contraction dim lands on SBUF
partitions with no on-device transposes.  Gate and backbone share the same
moving operand (xh^T); their transposed weight matrices are stacked into one
[768, 1024] tensor.  Matmul inputs are cast to fp16 on the host (halves the
input stream and runs the PE at full rate with fast weight load; fp16's
10-bit mantissa keeps matmul error ~4x below bf16 and the operands are far
from fp16 range limits); h is also
shipped in fp32 for the elementwise decay mix, which runs in fp32.
Elementwise uses the transposed layout where per-feature vectors (biases,
softplus(log_tau)) are per-partition scalars that fuse into ACT bias/scale
slots, and delta_t is broadcast along partitions once by a step-0 DMA.
sigmoid(u) is computed as 0.5 + 0.5*tanh(u/2) because Sigmoid and Exp never
share an ACT table (the 0.5s fold into existing affine slots); softplus(x) is
ln(1+exp(x)) for the same reason.  The [512, B_shard] per-core results are
gathered and transposed on the host.
"""

from contextlib import ExitStack

import ml_dtypes
import numpy as np

import concourse.bass as bass
import concourse.mybir as mybir
import concourse.tile as tile
from concourse import bacc
from concourse.bass_utils import run_bass_kernel_spmd

B, IN, H = 8192, 256, 512
NCORES = 8
BS = B // NCORES          # 1024 batch rows per core
KIN = IN + H              # 768 contraction dim
KT = KIN // 128           # 6 k-tiles
NJ = H // 128             # 4 partition tiles per output matrix
NCHUNK = 512              # matmul moving free dim per PSUM bank
NCH = BS // NCHUNK        # 2 b-chunks per core

F32 = mybir.dt.float32
F32R = mybir.dt.float32r
FP16 = mybir.dt.float16
AF = mybir.ActivationFunctionType
OP = mybir.AluOpType

USE_FP16 = True           # fp16 matmul inputs (2x PE rate + FWL) vs fp32r
TRACE = False             # test.py flips this for profiled runs
LAST_RESULT = None        # BassKernelResults of the most recent run

_NC_CACHE = None


def _body(tc, xhT, WT, hT, consts, negdt, outT):
    nc = tc.nc
    mm_dt = FP16 if USE_FP16 else F32
    with ExitStack() as ctx:
        singles = ctx.enter_context(tc.tile_pool(name="singles", bufs=1))
        work = ctx.enter_context(tc.tile_pool(name="work", bufs=3))
        psg = ctx.enter_context(tc.tile_pool(name="psg", bufs=2, space="PSUM"))
        psf = ctx.enter_context(tc.tile_pool(name="psf", bufs=2, space="PSUM"))

        # Persistent SBUF tensors
        xh_sb = singles.tile([128, KT, BS], mm_dt, tag="xh")
        w_sb = singles.tile([128, KT, 2 * H], mm_dt, tag="w")
        h_sb = singles.tile([128, NJ, BS], F32, tag="h")
        cst = singles.tile([128, 3, NJ], F32, tag="cst")
        stau = singles.tile([128, NJ], F32, tag="stau")
        ndt = singles.tile([128, BS], F32, tag="ndt")

        xh_v = xhT.rearrange("(k p) b -> k p b", p=128)
        w_v = WT.rearrange("(k p) n -> k p n", p=128)
        h_v = hT.rearrange("(j p) b -> j p b", p=128)

        # consts is [bg/2 | bb | log_tau] each [H]; column j of the SBUF tile
        # is that vector's j-th 128-row slice (per-partition scalars).
        nc.sync.dma_start(
            out=cst, in_=consts.rearrange("(c j p) -> p c j", p=128, j=NJ)
        )
        # -delta_t pre-broadcast on the host: a step-0 partition-broadcast DMA
        # emits 4-byte descriptors that monopolize all 16 DMA engines for the
        # whole kernel, so ship the materialized [128, BS] block instead.
        nc.sync.dma_start(out=ndt, in_=negdt)
        # softplus(log_tau) = ln(1 + exp(log_tau)); the ACT table with
        # Softplus itself never ships with Exp/Tanh, so build it from Ln/Exp.
        # stau holds softplus(log_tau) + 0.5 — the 0.5 is the constant term of
        # sigmoid(u) = 0.5 + 0.5*tanh(u/2), folded into tau below.
        e0 = singles.tile([128, NJ], F32, tag="e0")
        nc.scalar.activation(out=e0, in_=cst[:, 2, :], func=AF.Exp)
        nc.vector.tensor_scalar_add(e0, e0, 1.0)
        nc.scalar.activation(out=stau, in_=e0, func=AF.Ln)
        nc.vector.tensor_scalar_add(stau, stau, 0.5)

        # Stream inputs in matmul consumption order (k-major).
        def mm_cast(ap):
            return ap if USE_FP16 else ap.bitcast(F32R)

        # Merge input streams into few large DMAs (each ~512KB-1MB): the
        # HWDGE issue cost is per-instruction, and 4KB rows keep full BW.
        for kk in range(0, KT, 2):
            nc.sync.dma_start(
                out=mm_cast(w_sb[:, kk:kk + 2, :]),
                in_=mm_cast(w_v[kk:kk + 2].rearrange("k p n -> p k n")),
            )
            nc.sync.dma_start(
                out=mm_cast(xh_sb[:, kk:kk + 2, :]),
                in_=mm_cast(xh_v[kk:kk + 2].rearrange("k p b -> p k b")),
            )
        for jj in range(0, NJ, 2):
            nc.sync.dma_start(
                out=h_sb[:, jj:jj + 2, :],
                in_=h_v[jj:jj + 2].rearrange("j p b -> p j b"),
            )

        # Process output tiles in j-pairs: each j gets a [128, BS] 2-bank PSUM
        # accumulator (both b-chunks side by side), so the elementwise chain
        # runs on [128, BS] units — half the per-op overhead — while 2 pairs
        # x (gate+backbone) x 2 banks fill all 8 PSUM banks.
        for jh in range(NJ // 2):
            jpair = [2 * jh, 2 * jh + 1]
            zg = {j: psg.tile([128, BS], F32, tag="zg", name=f"zg_{j}") for j in jpair}
            zf = {j: psf.tile([128, BS], F32, tag="zf", name=f"zf_{j}") for j in jpair}
            for n in range(NCH):
                bsl = slice(n * NCHUNK, (n + 1) * NCHUNK)
                for k in range(KT):
                    rhs = mm_cast(xh_sb[:, k, bsl])
                    for j in jpair:
                        nc.tensor.matmul(
                            zg[j][:, bsl],
                            mm_cast(w_sb[:, k, j * 128:(j + 1) * 128]),
                            rhs,
                            start=(k == 0),
                            stop=(k == KT - 1),
                        )
                    for j in jpair:
                        nc.tensor.matmul(
                            zf[j][:, bsl],
                            mm_cast(w_sb[:, k, H + j * 128:H + (j + 1) * 128]),
                            rhs,
                            start=(k == 0),
                            stop=(k == KT - 1),
                        )
            for j in jpair:
                tg = work.tile([128, BS], F32, tag="tg", name=f"tg_{j}")
                f = work.tile([128, BS], F32, tag="f", name=f"f_{j}")
                tau = work.tile([128, BS], F32, tag="tau", name=f"tau_{j}")
                t = work.tile([128, BS], F32, tag="t", name=f"t_{j}")
                decay = work.tile([128, BS], F32, tag="decay", name=f"decay_{j}")
                hmf = work.tile([128, BS], F32, tag="hmf", name=f"hmf_{j}")
                p = work.tile([128, BS], F32, tag="p", name=f"p_{j}")
                o = work.tile([128, BS], F32, tag="o", name=f"o_{j}")

                # sigmoid(zg + bg) = 0.5 + 0.5*tanh((zg + bg)/2): Sigmoid never
                # shares an ACT table with Exp, but Tanh does.  cst slot 0
                # holds bg/2; the 0.5 offsets are folded into stau.
                nc.scalar.activation(
                    out=tg, in_=zg[j], func=AF.Tanh, bias=cst[:, 0, j:j + 1],
                    scale=0.5,
                )
                nc.scalar.activation(
                    out=f, in_=zf[j], func=AF.Tanh, bias=cst[:, 1, j:j + 1]
                )
                # tau = g + softplus(log_tau) = 0.5*tg + (softplus+0.5)
                nc.vector.tensor_scalar(
                    out=tau, in0=tg, scalar1=0.5, scalar2=stau[:, j:j + 1],
                    op0=OP.mult, op1=OP.add,
                )
                # t = -dt * tau
                nc.vector.tensor_mul(out=t, in0=tau, in1=ndt)
                nc.scalar.activation(out=decay, in_=t, func=AF.Exp)
                # out = f + decay * (h - f)
                nc.vector.tensor_sub(out=hmf, in0=h_sb[:, j, :], in1=f)
                nc.vector.tensor_mul(out=p, in0=decay, in1=hmf)
                nc.vector.tensor_add(out=o, in0=p, in1=f)
                nc.sync.dma_start(out=outT[j * 128:(j + 1) * 128, :], in_=o)


def build_nc():
    mm_np_dt = FP16 if USE_FP16 else F32
    nc = bacc.Bacc(
        "TRN2",
        target_bir_lowering=False,
        debug=False,
        enable_asserts=False,
        num_devices=NCORES,
    )
    xhT = nc.dram_tensor("xhT", [KIN, BS], mm_np_dt, kind="ExternalInput").ap()
    WT = nc.dram_tensor("WT", [KIN, 2 * H], mm_np_dt, kind="ExternalInput").ap()
    hT = nc.dram_tensor("hT", [H, BS], F32, kind="ExternalInput").ap()
    consts = nc.dram_tensor("consts", [3 * H], F32, kind="ExternalInput").ap()
    negdt = nc.dram_tensor("negdt", [128, BS], F32, kind="ExternalInput").ap()
    outT = nc.dram_tensor("outT", [H, BS], F32, kind="ExternalOutput").ap()
    with tile.TileContext(nc) as tc:
        _body(tc, xhT, WT, hT, consts, negdt, outT)
    nc.compile()
    return nc


def _get_nc():
    global _NC_CACHE
    if _NC_CACHE is None:
        _NC_CACHE = build_nc()
    return _NC_CACHE


def make_in_maps(x, h, delta_t, W_backbone, b_backbone, W_gx, b_gx, W_gh,
                 gate_b, log_tau):
    f32 = np.float32
    mm_dt = np.float16 if USE_FP16 else f32
    xh = np.concatenate(
        [np.asarray(x, f32), np.asarray(h, f32)], axis=1
    )                                                   # [B, 768]
    xhT = np.ascontiguousarray(xh.T)                    # [768, B]
    hT = xhT[IN:]                                       # [512, B] fp32 view
    WgT = np.concatenate(
        [np.asarray(W_gx, f32), np.asarray(W_gh, f32)], axis=1
    ).T                                                 # [768, H]
    WT = np.concatenate(
        [WgT, np.asarray(W_backbone, f32).T], axis=1
    ).astype(mm_dt)                                     # [768, 2H]
    WT = np.ascontiguousarray(WT)
    xhT_mm = xhT.astype(mm_dt)
    consts = np.concatenate(
        [
            (np.asarray(b_gx, f32) + np.asarray(gate_b, f32)) * 0.5,
            np.asarray(b_backbone, f32),
            np.asarray(log_tau, f32),
        ]
    ).astype(f32)                                       # [3H]
    negdt = (-np.asarray(delta_t, f32)).astype(f32)     # [B]

    in_maps = []
    for c in range(NCORES):
        sl = slice(c * BS, (c + 1) * BS)
        in_maps.append(
            {
                "xhT": np.ascontiguousarray(xhT_mm[:, sl]),
                "WT": WT,
                "hT": np.ascontiguousarray(hT[:, sl]),
                "consts": consts,
                "negdt": np.ascontiguousarray(np.broadcast_to(negdt[sl][None, :], (128, BS))),
            }
        )
    return in_maps


def kernel(x, h, delta_t, W_backbone, b_backbone, W_gx, b_gx, W_gh, gate_b,
           log_tau):
    global LAST_RESULT
    in_maps = make_in_maps(x, h, delta_t, W_backbone, b_backbone, W_gx, b_gx,
                           W_gh, gate_b, log_tau)
    nc = _get_nc()
    res = run_bass_kernel_spmd(
        nc, in_maps, core_ids=list(range(NCORES)), trace=TRACE
    )
    LAST_RESULT = res
    out = np.concatenate([r["outT"] for r in res.results], axis=1).T
    return np.ascontiguousarray(out).astype(np.float32)
